# revision 16
# baseline (speedup 1.0000x reference)
"""ConsistencyLoss kernel for 8 TRN2 NeuronCores (Bass/Tile).

loss = mean_b mean_{j,k} | |m1_j - m1_k| - |m2_j - m2_k| |
  m1 = per-segment means of channel-mean(input)
  m2 = per-segment means of channel-mean(bilinear_up(feature))

Sharding: data-parallel over batch B=8, one batch element per core.
Per-core pipeline (v2):
  - channel-mean of input on DVE via bf16 in-place add tree (h on partitions)
  - feature: channel-mean via ones-matmul, separable bilinear upsample via two
    small matmuls with host-built interpolation matrices
  - segment reduction, S=256=16*16 hi/lo one-hot decomposition, grouped
    cross-matrix matmuls: 8 pixel-columns share one [128,128] stationary
    (oh_hi), one N=384 moving block [A1|A2|oh_lo]x8 accumulates into a
    [128,384] PSUM tile; only the 8 diagonal [16,48] blocks are read out.
  - similarity-matrix L1 via K=2 outer-product matmuls + DVE abs/reduce
Host: casts sp to bf16 lo/hi planes, builds interp matrices, averages the
8 per-core sums.
"""

import sys

if "/opt/trn_rl_repo" not in sys.path:
    sys.path.insert(0, "/opt/trn_rl_repo")

import numpy as np
import ml_dtypes

import concourse.bacc as bacc
import concourse.mybir as mybir
import concourse.tile as tile
from concourse.bass_utils import run_bass_kernel_spmd

B, C, H, W = 8, 64, 256, 256
FH, FW = 64, 64
S = 256
N_CORES = 8

F32 = mybir.dt.float32
BF16 = mybir.dt.bfloat16

_CACHE = {}


def _interp_matrix(out_size: int, in_size: int) -> np.ndarray:
    """R [out,in]: bilinear align_corners row-interp matrix (float32)."""
    r = np.zeros((out_size, in_size), dtype=np.float64)
    ys = np.linspace(0.0, in_size - 1.0, out_size)
    y0 = np.floor(ys).astype(np.int64)
    y1 = np.minimum(y0 + 1, in_size - 1)
    wy = ys - y0
    for o in range(out_size):
        r[o, y0[o]] += 1.0 - wy[o]
        r[o, y1[o]] += wy[o]
    return r.astype(np.float32)


def _build_nc(dbg=False):
    nc = bacc.Bacc("TRN2", target_bir_lowering=False, debug=False,
                   num_devices=N_CORES)

    x = nc.dram_tensor("x", [C, H, W], F32, kind="ExternalInput").ap()
    f = nc.dram_tensor("f", [C, FH * FW], BF16, kind="ExternalInput").ap()
    lo = nc.dram_tensor("lo", [H, W], BF16, kind="ExternalInput").ap()
    hi = nc.dram_tensor("hi", [H, W], BF16, kind="ExternalInput").ap()
    iota16 = nc.dram_tensor("iota16", [128, 16], BF16, kind="ExternalInput").ap()
    ryt = nc.dram_tensor("ryt", [FH, H], F32, kind="ExternalInput").ap()
    rxt = nc.dram_tensor("rxt", [FW, W], F32, kind="ExternalInput").ap()
    ones64 = nc.dram_tensor("ones64", [C, 1], BF16, kind="ExternalInput").ap()
    ones128 = nc.dram_tensor("ones128", [128, 1], F32, kind="ExternalInput").ap()
    onesrow = nc.dram_tensor("onesrow", [1, S], F32, kind="ExternalInput").ap()
    out = nc.dram_tensor("out", [1, 1], F32, kind="ExternalOutput").ap()
    if dbg:
        dbg_acc = nc.dram_tensor("dbg_acc", [16, 48], F32, kind="ExternalOutput").ap()
        dbg_px = nc.dram_tensor("dbg_px", [128, W], F32, kind="ExternalOutput").ap()
        dbg_m1 = nc.dram_tensor("dbg_m1", [16, 16], F32, kind="ExternalOutput").ap()
        dbg_m2 = nc.dram_tensor("dbg_m2", [16, 16], F32, kind="ExternalOutput").ap()

    CCH = 32          # channels per input DMA chunk
    NCC = C // CCH
    GW = 8            # pixel-columns per stationary group
    NG = W // GW      # groups per h-block

    with tile.TileContext(nc) as tc:
        with (
            tc.tile_pool(name="const", bufs=1) as const,
            tc.tile_pool(name="xin", bufs=2) as xin,
            tc.tile_pool(name="tree", bufs=1) as treep,
            tc.tile_pool(name="mov", bufs=2) as movp,
            tc.tile_pool(name="ohp", bufs=2) as ohp,
            tc.tile_pool(name="work", bufs=2) as work,
            tc.tile_pool(name="small", bufs=2) as small,
            tc.tile_pool(name="tail", bufs=1) as tailp,
            tc.tile_pool(name="ps1", bufs=1, space="PSUM") as ps1,
            tc.tile_pool(name="fmp", bufs=2, space="PSUM") as fmp,
            tc.tile_pool(name="psacc", bufs=1, space="PSUM") as psacc,
            tc.tile_pool(name="dps", bufs=1, space="PSUM") as dps,
        ):
            # ---- constants ----
            iota_sb = const.tile([128, 16], BF16, tag="iota")
            nc.sync.dma_start(iota_sb[:], iota16[:])
            ryt_sb = const.tile([FH, H], F32, tag="ryt")
            nc.sync.dma_start(ryt_sb[:], ryt[:])
            rxt_sb = const.tile([FW, W], F32, tag="rxt")
            nc.sync.dma_start(rxt_sb[:], rxt[:])
            ones64_sb = const.tile([C, 1], BF16, tag="o64")
            nc.sync.dma_start(ones64_sb[:], ones64[:])
            ones128_sb = const.tile([128, 1], F32, tag="o128")
            nc.sync.dma_start(ones128_sb[:], ones128[:])

            # ---- feature path: channel mean -> fm [64 h', 64 w'] ----
            fsb = const.tile([C, FH * FW], BF16, tag="fsb")
            nc.sync.dma_start(fsb[:], f[:])
            fmsb = const.tile([FH, FW], F32, tag="fmsb")
            for i in range(8):
                fm_ps = fmp.tile([1, 512], F32, tag="fmps")
                nc.tensor.matmul(fm_ps[:], ones64_sb[:], fsb[:, i * 512:(i + 1) * 512])
                fmpart = small.tile([1, 512], F32, tag="fmpart")
                nc.scalar.copy(fmpart[:], fm_ps[:])
                nc.sync.dma_start(fmsb[i * 8:(i + 1) * 8, :], fmpart[:])

            # ---- bilinear upsample: fmup = Ry @ fm @ Rx^T ----
            t1_ps = ps1.tile([FW, H], F32, tag="t1ps")
            nc.tensor.matmul(t1_ps[:], fmsb[:], ryt_sb[:])
            t1_sb = const.tile([FW, H], F32, tag="t1sb")
            nc.scalar.copy(t1_sb[:], t1_ps[:])

            px2bf = []
            for hb in range(2):
                up_ps = ps1.tile([128, W], F32, tag="upps")
                nc.tensor.matmul(up_ps[:], t1_sb[:, hb * 128:(hb + 1) * 128],
                                 rxt_sb[:])
                p2 = work.tile([128, W], BF16, tag=f"px2bf{hb}")
                nc.scalar.copy(p2[:], up_ps[:])
                px2bf.append(p2)

            # ---- main loop over h-blocks ----
            acc_ps = psacc.tile([128, 3 * 128], F32, tag="acc")
            for hb in range(2):
                # input channel-sum via accumulate-DMA (CCE adds in DMA path):
                # 2 tiles x 4 overlaid 8-channel slabs -> [128, 8, 256] each,
                # then a small bf16 tree on DVE.
                halves = []
                for t in range(2):
                    xt = xin.tile([128, 8, W], F32, tag=f"xt{t}")
                    c0 = t * 32
                    for d in range(4):
                        src = x[c0 + d * 8:c0 + (d + 1) * 8,
                                hb * 128:(hb + 1) * 128, :].rearrange(
                                    "c h w -> h c w")
                        nc.gpsimd.dma_start(
                            xt[:], src,
                            accum_op=(mybir.AluOpType.bypass if d == 0
                                      else mybir.AluOpType.add))
                    t4 = treep.tile([128, 4, W], BF16, tag=f"t4_{t}")
                    nc.vector.tensor_add(t4[:], xt[:, 0:4, :], xt[:, 4:8, :])
                    nc.vector.tensor_add(t4[:, 0:2, :], t4[:, 0:2, :],
                                         t4[:, 2:4, :])
                    nc.vector.tensor_add(t4[:, 0:1, :], t4[:, 0:1, :],
                                         t4[:, 1:2, :])
                    halves.append(t4)
                px1 = work.tile([128, W], BF16, tag="px1bf")
                nc.vector.tensor_add(px1[:].unsqueeze(1), halves[0][:, 0:1, :],
                                     halves[1][:, 0:1, :])
                if dbg and hb == 0:
                    pxf = tailp.tile([128, W], F32, tag="dbgpx")
                    nc.vector.tensor_copy(pxf[:], px1[:])
                    nc.sync.dma_start(dbg_px[:], pxf[:])

                # segment ids
                lo_sb = work.tile([128, W], BF16, tag="losb")
                nc.sync.dma_start(lo_sb[:], lo[hb * 128:(hb + 1) * 128, :])
                hi_sb = work.tile([128, W], BF16, tag="hisb")
                nc.sync.dma_start(hi_sb[:], hi[hb * 128:(hb + 1) * 128, :])

                # one-hots + A tiles in (w, j) layout
                mov = movp.tile([128, W, 48], BF16, tag="mov")
                ohhi = ohp.tile([128, W, 16], BF16, tag="ohhi")
                iota_b = iota_sb[:, :].unsqueeze(1).to_broadcast([128, W, 16])
                lo_b = lo_sb[:, :].unsqueeze(2).to_broadcast([128, W, 16])
                hi_b = hi_sb[:, :].unsqueeze(2).to_broadcast([128, W, 16])
                px1_b = px1[:, :].unsqueeze(2).to_broadcast([128, W, 16])
                px2_b = px2bf[hb][:, :].unsqueeze(2).to_broadcast([128, W, 16])
                ohlo = mov[:, :, 32:48]
                nc.vector.tensor_tensor(ohlo, iota_b, lo_b,
                                        op=mybir.AluOpType.is_equal)
                nc.vector.tensor_tensor(ohhi[:], iota_b, hi_b,
                                        op=mybir.AluOpType.is_equal)
                nc.vector.tensor_tensor(mov[:, :, 0:16], ohlo, px1_b,
                                        op=mybir.AluOpType.mult)
                nc.vector.tensor_tensor(mov[:, :, 16:32], ohlo, px2_b,
                                        op=mybir.AluOpType.mult)

                # grouped segment matmuls: one N=384 MM per 8 columns
                for g in range(NG):
                    lhs = ohhi[:, g * GW:(g + 1) * GW, :].rearrange(
                        "p w j -> p (w j)")
                    rhs = mov[:, g * GW:(g + 1) * GW, :].rearrange(
                        "p w j -> p (w j)")
                    nc.tensor.matmul(acc_ps[:], lhs, rhs,
                                     start=(hb == 0 and g == 0),
                                     stop=(hb == 1 and g == NG - 1))

            # ---- diagonal extraction: 8 blocks [16, 48] -> summed [16, 48] ----
            acc_all = tailp.tile([128, 3 * 128], F32, tag="accall")
            nc.scalar.copy(acc_all[:], acc_ps[:])
            diagt = small.tile([16, GW, 48], F32, tag="diagt")
            for i in range(GW):
                nc.sync.dma_start(
                    diagt[:, i, :],
                    acc_all[16 * i:16 * (i + 1), 48 * i:48 * (i + 1)])
            acc_sb = small.tile([16, 48], F32, tag="accsb")
            nc.vector.tensor_reduce(
                acc_sb[:], diagt.rearrange("p s j -> p j s"),
                axis=mybir.AxisListType.X, op=mybir.AluOpType.add)
            if dbg:
                nc.sync.dma_start(dbg_acc[:], acc_sb[:])

            # ---- m1/m2 [16,16] ----
            cntm = small.tile([16, 16], F32, tag="cntm")
            nc.vector.tensor_scalar_max(cntm[:], acc_sb[:, 32:48], 0.5)
            rc2 = small.tile([16, 16], F32, tag="rc2")
            nc.vector.reciprocal(rc2[:], cntm[:])
            rc1 = small.tile([16, 16], F32, tag="rc1")
            nc.vector.tensor_scalar_mul(rc1[:], rc2[:], 1.0 / C)
            m1 = small.tile([16, 16], F32, tag="m1")
            nc.vector.tensor_tensor(m1[:], acc_sb[:, 0:16], rc1[:],
                                    op=mybir.AluOpType.mult)
            m2 = small.tile([16, 16], F32, tag="m2")
            nc.vector.tensor_tensor(m2[:], acc_sb[:, 16:32], rc2[:],
                                    op=mybir.AluOpType.mult)
            if dbg:
                nc.sync.dma_start(dbg_m1[:], m1[:])
                nc.sync.dma_start(dbg_m2[:], m2[:])

            # rows+ones tiles for outer-product difference matmuls:
            # d[j,k] = m_j - m_k = [m_block; 1]^T @ [[1...]; [-m_row]]
            lhs_m, rhs_m = [], []
            for (msrc, nm) in ((m1, "m1"), (m2, "m2")):
                mneg = small.tile([16, 16], F32, tag=f"mneg{nm}")
                nc.vector.tensor_scalar_mul(mneg[:], msrc[:], -1.0)
                lh = small.tile([2, S], F32, tag=f"lh{nm}")
                nc.sync.dma_start(lh[0:1, :], msrc[:, :])
                nc.sync.dma_start(lh[1:2, :], onesrow[:])
                rh = small.tile([2, S], F32, tag=f"rh{nm}")
                nc.sync.dma_start(rh[0:1, :], onesrow[:])
                nc.sync.dma_start(rh[1:2, :], mneg[:, :])
                lhs_m.append(lh)
                rhs_m.append(rh)

            # ---- loss: sum_{j,k} ||m1_j-m1_k| - |m2_j-m2_k|| ----
            total = small.tile([128, 1], F32, tag="total")
            for jb in range(2):
                dps1 = dps.tile([128, S], F32, tag="dps1")
                nc.tensor.matmul(dps1[:], lhs_m[0][:, jb * 128:(jb + 1) * 128],
                                 rhs_m[0][:])
                dps2 = dps.tile([128, S], F32, tag="dps2")
                nc.tensor.matmul(dps2[:], lhs_m[1][:, jb * 128:(jb + 1) * 128],
                                 rhs_m[1][:])
                d1 = tailp.tile([128, S], F32, tag="d1")
                nc.vector.tensor_scalar(
                    d1[:].bitcast(mybir.dt.int32),
                    dps1[:].bitcast(mybir.dt.int32), 0x7FFFFFFF, None,
                    op0=mybir.AluOpType.bitwise_and)
                d2 = tailp.tile([128, S], F32, tag="d2")
                nc.vector.tensor_scalar(
                    d2[:].bitcast(mybir.dt.int32),
                    dps2[:].bitcast(mybir.dt.int32), 0x7FFFFFFF, None,
                    op0=mybir.AluOpType.bitwise_and)
                dd = tailp.tile([128, S], F32, tag="dd")
                nc.vector.tensor_tensor(dd[:], d1[:], d2[:],
                                        op=mybir.AluOpType.subtract)
                part = small.tile([128, 1], F32, tag=f"part{jb}")
                nc.vector.tensor_reduce(
                    part[:], dd[:], axis=mybir.AxisListType.X,
                    op=mybir.AluOpType.add, apply_absolute_value=True)
                if jb == 0:
                    tot0 = part
                else:
                    nc.vector.tensor_add(total[:], tot0[:], part[:])

            loss_ps = ps1.tile([1, 1], F32, tag="t1ps")
            nc.tensor.matmul(loss_ps[:], ones128_sb[:], total[:])
            loss_sb = small.tile([1, 1], F32, tag="losssb")
            nc.vector.tensor_copy(loss_sb[:], loss_ps[:])
            nc.sync.dma_start(out[:], loss_sb[:])

    nc.compile()
    return nc


def _get_nc():
    if "nc" not in _CACHE:
        _CACHE["nc"] = _build_nc()
    return _CACHE["nc"]


def _host_inputs(input, feature, sp):
    sp32 = np.asarray(sp).astype(np.int32).reshape(B, H, W)
    lo = (sp32 & 15).astype(ml_dtypes.bfloat16)
    hi = (sp32 >> 4).astype(ml_dtypes.bfloat16)
    iota16 = np.broadcast_to(
        np.arange(16, dtype=np.float32)[None, :], (128, 16)
    ).astype(ml_dtypes.bfloat16)
    iota16 = np.ascontiguousarray(iota16)
    ryt = np.ascontiguousarray(_interp_matrix(H, FH).T)   # [64, 256]
    rxt = np.ascontiguousarray(_interp_matrix(W, FW).T)   # [64, 256]
    ones64 = np.full((C, 1), 1.0 / C, dtype=ml_dtypes.bfloat16)
    ones128 = np.ones((128, 1), dtype=np.float32)
    onesrow = np.ones((1, S), dtype=np.float32)
    xf = np.ascontiguousarray(np.asarray(input, dtype=np.float32))
    ff = np.ascontiguousarray(
        np.asarray(feature, dtype=np.float32)
        .reshape(B, C, FH * FW).astype(ml_dtypes.bfloat16))
    in_maps = []
    for b in range(B):
        in_maps.append({
            "x": xf[b],
            "f": ff[b],
            "lo": np.ascontiguousarray(lo[b]),
            "hi": np.ascontiguousarray(hi[b]),
            "iota16": iota16,
            "ryt": ryt,
            "rxt": rxt,
            "ones64": ones64,
            "ones128": ones128,
            "onesrow": onesrow,
        })
    return in_maps


def _run(inputs, trace=False, **kw):
    nc = _get_nc()
    in_maps = _host_inputs(inputs["input"], inputs["feature"], inputs["sp"])
    res = run_bass_kernel_spmd(nc, in_maps, core_ids=list(range(N_CORES)),
                               trace=trace, **kw)
    sums = np.array([res.results[i]["out"][0, 0] for i in range(N_CORES)],
                    dtype=np.float64)
    loss = (sums / float(S * S)).mean()
    return np.float32(loss), res


def kernel(**inputs) -> np.ndarray:
    loss, _ = _run(inputs, trace=False)
    return np.asarray(loss, dtype=np.float32)


# revision 18
# speedup vs baseline: 1.1128x; 1.1128x over previous
"""ConsistencyLoss kernel for 8 TRN2 NeuronCores (Bass/Tile).

loss = mean_b mean_{j,k} | |m1_j - m1_k| - |m2_j - m2_k| |
  m1 = per-segment means of channel-mean(input)
  m2 = per-segment means of channel-mean(bilinear_up(feature))

Sharding: data-parallel over batch B=8, one batch element per core.
Per-core pipeline (v2):
  - channel-mean of input on DVE via bf16 in-place add tree (h on partitions)
  - feature: channel-mean via ones-matmul, separable bilinear upsample via two
    small matmuls with host-built interpolation matrices
  - segment reduction, S=256=16*16 hi/lo one-hot decomposition, grouped
    cross-matrix matmuls: 8 pixel-columns share one [128,128] stationary
    (oh_hi), one N=384 moving block [A1|A2|oh_lo]x8 accumulates into a
    [128,384] PSUM tile; only the 8 diagonal [16,48] blocks are read out.
  - similarity-matrix L1 via K=2 outer-product matmuls + DVE abs/reduce
Host: casts sp to bf16 lo/hi planes, builds interp matrices, averages the
8 per-core sums.
"""

import sys

if "/opt/trn_rl_repo" not in sys.path:
    sys.path.insert(0, "/opt/trn_rl_repo")

import numpy as np
import ml_dtypes

import concourse.bacc as bacc
import concourse.mybir as mybir
import concourse.tile as tile
from concourse.bass_utils import run_bass_kernel_spmd

B, C, H, W = 8, 64, 256, 256
FH, FW = 64, 64
S = 256
N_CORES = 8

F32 = mybir.dt.float32
BF16 = mybir.dt.bfloat16

_CACHE = {}


def _interp_matrix(out_size: int, in_size: int) -> np.ndarray:
    """R [out,in]: bilinear align_corners row-interp matrix (float32)."""
    r = np.zeros((out_size, in_size), dtype=np.float64)
    ys = np.linspace(0.0, in_size - 1.0, out_size)
    y0 = np.floor(ys).astype(np.int64)
    y1 = np.minimum(y0 + 1, in_size - 1)
    wy = ys - y0
    for o in range(out_size):
        r[o, y0[o]] += 1.0 - wy[o]
        r[o, y1[o]] += wy[o]
    return r.astype(np.float32)


def _build_nc(dbg=False):
    nc = bacc.Bacc("TRN2", target_bir_lowering=False, debug=False,
                   num_devices=N_CORES)

    x = nc.dram_tensor("x", [C, H, W], F32, kind="ExternalInput").ap()
    f = nc.dram_tensor("f", [C, FH * FW], BF16, kind="ExternalInput").ap()
    lo = nc.dram_tensor("lo", [H, W], BF16, kind="ExternalInput").ap()
    ohhi_in = nc.dram_tensor("ohhi", [H, W * 16], BF16, kind="ExternalInput").ap()
    iota16 = nc.dram_tensor("iota16", [128, 16], BF16, kind="ExternalInput").ap()
    ryt = nc.dram_tensor("ryt", [FH, H], F32, kind="ExternalInput").ap()
    rxt = nc.dram_tensor("rxt", [FW, W], F32, kind="ExternalInput").ap()
    ones64 = nc.dram_tensor("ones64", [C, 1], BF16, kind="ExternalInput").ap()
    ones128 = nc.dram_tensor("ones128", [128, 1], F32, kind="ExternalInput").ap()
    onesrow = nc.dram_tensor("onesrow", [1, S], F32, kind="ExternalInput").ap()
    out = nc.dram_tensor("out", [1, 1], F32, kind="ExternalOutput").ap()
    if dbg:
        dbg_acc = nc.dram_tensor("dbg_acc", [16, 48], F32, kind="ExternalOutput").ap()
        dbg_px = nc.dram_tensor("dbg_px", [128, W], F32, kind="ExternalOutput").ap()
        dbg_m1 = nc.dram_tensor("dbg_m1", [16, 16], F32, kind="ExternalOutput").ap()
        dbg_m2 = nc.dram_tensor("dbg_m2", [16, 16], F32, kind="ExternalOutput").ap()

    CCH = 32          # channels per input DMA chunk
    NCC = C // CCH
    GW = 8            # pixel-columns per stationary group
    NG = W // GW      # groups per h-block

    with tile.TileContext(nc) as tc:
        with (
            tc.tile_pool(name="const", bufs=1) as const,
            tc.tile_pool(name="xin", bufs=2) as xin,
            tc.tile_pool(name="tree", bufs=1) as treep,
            tc.tile_pool(name="mov", bufs=2) as movp,
            tc.tile_pool(name="ohp", bufs=2) as ohp,
            tc.tile_pool(name="work", bufs=2) as work,
            tc.tile_pool(name="small", bufs=2) as small,
            tc.tile_pool(name="tail", bufs=1) as tailp,
            tc.tile_pool(name="ps1", bufs=1, space="PSUM") as ps1,
            tc.tile_pool(name="fmp", bufs=2, space="PSUM") as fmp,
            tc.tile_pool(name="psacc", bufs=1, space="PSUM") as psacc,
            tc.tile_pool(name="dps", bufs=1, space="PSUM") as dps,
        ):
            # ---- constants ----
            iota_sb = const.tile([128, 16], BF16, tag="iota")
            nc.sync.dma_start(iota_sb[:], iota16[:])
            ryt_sb = const.tile([FH, H], F32, tag="ryt")
            nc.sync.dma_start(ryt_sb[:], ryt[:])
            rxt_sb = const.tile([FW, W], F32, tag="rxt")
            nc.sync.dma_start(rxt_sb[:], rxt[:])
            ones64_sb = const.tile([C, 1], BF16, tag="o64")
            nc.sync.dma_start(ones64_sb[:], ones64[:])
            ones128_sb = const.tile([128, 1], F32, tag="o128")
            nc.sync.dma_start(ones128_sb[:], ones128[:])

            # lh/rh ones-halves prefilled early
            lhrh = {}
            for nm in ("m1", "m2"):
                lh = const.tile([2, S], F32, tag=f"lh{nm}")
                nc.sync.dma_start(lh[1:2, :], onesrow[:])
                rh = const.tile([2, S], F32, tag=f"rh{nm}")
                nc.sync.dma_start(rh[0:1, :], onesrow[:])
                lhrh[nm] = (lh, rh)

            # ---- feature path: channel mean -> fm [64 h', 64 w'] ----
            fsb = const.tile([C, FH * FW], BF16, tag="fsb")
            nc.sync.dma_start(fsb[:], f[:])
            fmsb = const.tile([FH, FW], F32, tag="fmsb")
            for i in range(8):
                fm_ps = fmp.tile([1, 512], F32, tag="fmps")
                nc.tensor.matmul(fm_ps[:], ones64_sb[:], fsb[:, i * 512:(i + 1) * 512])
                fmpart = small.tile([1, 512], F32, tag="fmpart")
                nc.scalar.copy(fmpart[:], fm_ps[:])
                nc.sync.dma_start(fmsb[i * 8:(i + 1) * 8, :], fmpart[:])

            # ---- bilinear upsample: fmup = Ry @ fm @ Rx^T ----
            t1_ps = ps1.tile([FW, H], F32, tag="t1ps")
            nc.tensor.matmul(t1_ps[:], fmsb[:], ryt_sb[:])
            t1_sb = const.tile([FW, H], F32, tag="t1sb")
            nc.scalar.copy(t1_sb[:], t1_ps[:])

            px2bf = []
            for hb in range(2):
                up_ps = ps1.tile([128, W], F32, tag="upps")
                nc.tensor.matmul(up_ps[:], t1_sb[:, hb * 128:(hb + 1) * 128],
                                 rxt_sb[:])
                p2 = work.tile([128, W], BF16, tag=f"px2bf{hb}")
                nc.scalar.copy(p2[:], up_ps[:])
                px2bf.append(p2)

            # ---- main loop over h-blocks ----
            acc_ps = psacc.tile([128, 3 * 128], F32, tag="acc")
            for hb in range(2):
                # input channel-sum via accumulate-DMA (CCE adds in DMA path):
                # 2 tiles x 4 overlaid 8-channel slabs -> [128, 8, 256] each,
                # then a small bf16 tree on DVE.
                halves = []
                for cc in range(NCC):
                    xt = xin.tile([128, CCH, W], F32, tag="xt")
                    nc.sync.dma_start(
                        xt[:],
                        x[cc * CCH:(cc + 1) * CCH,
                          hb * 128:(hb + 1) * 128, :].rearrange("c h w -> h c w"),
                    )
                    t16 = treep.tile([128, 16, W], BF16, tag=f"t16_{cc}")
                    nc.vector.tensor_add(t16[:], xt[:, 0:16, :], xt[:, 16:32, :])
                    nc.vector.tensor_add(t16[:, 0:8, :], t16[:, 0:8, :],
                                         t16[:, 8:16, :])
                    nc.vector.tensor_add(t16[:, 0:4, :], t16[:, 0:4, :],
                                         t16[:, 4:8, :])
                    nc.vector.tensor_add(t16[:, 0:2, :], t16[:, 0:2, :],
                                         t16[:, 2:4, :])
                    nc.vector.tensor_add(t16[:, 0:1, :], t16[:, 0:1, :],
                                         t16[:, 1:2, :])
                    halves.append(t16)
                px1 = work.tile([128, W], BF16, tag="px1bf")
                nc.vector.tensor_add(px1[:].unsqueeze(1), halves[0][:, 0:1, :],
                                     halves[1][:, 0:1, :])
                if dbg and hb == 0:
                    pxf = tailp.tile([128, W], F32, tag="dbgpx")
                    nc.vector.tensor_copy(pxf[:], px1[:])
                    nc.sync.dma_start(dbg_px[:], pxf[:])

                # segment ids + prebuilt hi one-hot
                lo_sb = work.tile([128, W], BF16, tag="losb")
                nc.sync.dma_start(lo_sb[:], lo[hb * 128:(hb + 1) * 128, :])
                ohhi = ohp.tile([128, W, 16], BF16, tag="ohhi")
                nc.sync.dma_start(
                    ohhi[:],
                    ohhi_in[hb * 128:(hb + 1) * 128, :].rearrange(
                        "p (w j) -> p w j", j=16))

                # lo one-hot + A tiles in (w, j) layout, built per w-half so
                # the grouped matmuls overlap the builds
                mov = movp.tile([128, W, 48], BF16, tag="mov")
                ohlo = mov[:, :, 32:48]
                WH = W // 2
                for wh in range(2):
                    ws = slice(wh * WH, (wh + 1) * WH)
                    iota_b = iota_sb[:, :].unsqueeze(1).to_broadcast(
                        [128, WH, 16])
                    lo_b = lo_sb[:, ws].unsqueeze(2).to_broadcast(
                        [128, WH, 16])
                    px1_b = px1[:, ws].unsqueeze(2).to_broadcast(
                        [128, WH, 16])
                    px2_b = px2bf[hb][:, ws].unsqueeze(2).to_broadcast(
                        [128, WH, 16])
                    nc.vector.tensor_tensor(mov[:, ws, 32:48], iota_b, lo_b,
                                            op=mybir.AluOpType.is_equal)
                    nc.vector.tensor_tensor(mov[:, ws, 0:16],
                                            mov[:, ws, 32:48], px1_b,
                                            op=mybir.AluOpType.mult)
                    nc.vector.tensor_tensor(mov[:, ws, 16:32],
                                            mov[:, ws, 32:48], px2_b,
                                            op=mybir.AluOpType.mult)
                    ng2 = NG // 2
                    for g in range(wh * ng2, (wh + 1) * ng2):
                        lhs = ohhi[:, g * GW:(g + 1) * GW, :].rearrange(
                            "p w j -> p (w j)")
                        rhs = mov[:, g * GW:(g + 1) * GW, :].rearrange(
                            "p w j -> p (w j)")
                        nc.tensor.matmul(
                            acc_ps[:], lhs, rhs,
                            start=(hb == 0 and g == 0),
                            stop=(hb == 1 and g == NG - 1))

            # ---- diagonal extraction: 8 blocks [16, 48] -> summed [16, 48] ----
            acc_all = tailp.tile([128, 3 * 128], F32, tag="accall")
            nc.scalar.copy(acc_all[:], acc_ps[:])
            diagt = small.tile([16, GW, 48], F32, tag="diagt")
            for i in range(GW):
                nc.sync.dma_start(
                    diagt[:, i, :],
                    acc_all[16 * i:16 * (i + 1), 48 * i:48 * (i + 1)])
            acc_sb = small.tile([16, 48], F32, tag="accsb")
            nc.vector.tensor_reduce(
                acc_sb[:], diagt.rearrange("p s j -> p j s"),
                axis=mybir.AxisListType.X, op=mybir.AluOpType.add)
            if dbg:
                nc.sync.dma_start(dbg_acc[:], acc_sb[:])

            # ---- m1/m2 [16,16] ----
            cntm = small.tile([16, 16], F32, tag="cntm")
            nc.vector.tensor_scalar_max(cntm[:], acc_sb[:, 32:48], 0.5)
            rc2 = small.tile([16, 16], F32, tag="rc2")
            nc.vector.reciprocal(rc2[:], cntm[:])
            rc1 = small.tile([16, 16], F32, tag="rc1")
            nc.vector.tensor_scalar_mul(rc1[:], rc2[:], 1.0 / C)
            m1 = small.tile([16, 16], F32, tag="m1")
            nc.vector.tensor_tensor(m1[:], acc_sb[:, 0:16], rc1[:],
                                    op=mybir.AluOpType.mult)
            m2 = small.tile([16, 16], F32, tag="m2")
            nc.vector.tensor_tensor(m2[:], acc_sb[:, 16:32], rc2[:],
                                    op=mybir.AluOpType.mult)
            if dbg:
                nc.sync.dma_start(dbg_m1[:], m1[:])
                nc.sync.dma_start(dbg_m2[:], m2[:])

            # rows+ones tiles for outer-product difference matmuls:
            # d[j,k] = m_j - m_k = [m_block; 1]^T @ [[1...]; [-m_row]]
            lhs_m, rhs_m = [], []
            for (msrc, nm) in ((m1, "m1"), (m2, "m2")):
                mneg = small.tile([16, 16], F32, tag=f"mneg{nm}")
                nc.vector.tensor_scalar_mul(mneg[:], msrc[:], -1.0)
                nc.sync.dma_start(lhrh[nm][0][0:1, :], msrc[:, :])
                nc.sync.dma_start(lhrh[nm][1][1:2, :], mneg[:, :])
                lhs_m.append(lhrh[nm][0])
                rhs_m.append(lhrh[nm][1])

            # ---- loss: sum_{j,k} ||m1_j-m1_k| - |m2_j-m2_k|| ----
            total = small.tile([128, 1], F32, tag="total")
            for jb in range(2):
                dps1 = dps.tile([128, S], F32, tag="dps1")
                nc.tensor.matmul(dps1[:], lhs_m[0][:, jb * 128:(jb + 1) * 128],
                                 rhs_m[0][:])
                dps2 = dps.tile([128, S], F32, tag="dps2")
                nc.tensor.matmul(dps2[:], lhs_m[1][:, jb * 128:(jb + 1) * 128],
                                 rhs_m[1][:])
                d1 = tailp.tile([128, S], F32, tag="d1")
                nc.vector.tensor_scalar(
                    d1[:].bitcast(mybir.dt.int32),
                    dps1[:].bitcast(mybir.dt.int32), 0x7FFFFFFF, None,
                    op0=mybir.AluOpType.bitwise_and)
                d2 = tailp.tile([128, S], F32, tag="d2")
                nc.vector.tensor_scalar(
                    d2[:].bitcast(mybir.dt.int32),
                    dps2[:].bitcast(mybir.dt.int32), 0x7FFFFFFF, None,
                    op0=mybir.AluOpType.bitwise_and)
                dd = tailp.tile([128, S], F32, tag="dd")
                nc.vector.tensor_tensor(dd[:], d1[:], d2[:],
                                        op=mybir.AluOpType.subtract)
                part = small.tile([128, 1], F32, tag=f"part{jb}")
                nc.vector.tensor_reduce(
                    part[:], dd[:], axis=mybir.AxisListType.X,
                    op=mybir.AluOpType.add, apply_absolute_value=True)
                if jb == 0:
                    tot0 = part
                else:
                    nc.vector.tensor_add(total[:], tot0[:], part[:])

            loss_ps = ps1.tile([1, 1], F32, tag="t1ps")
            nc.tensor.matmul(loss_ps[:], ones128_sb[:], total[:])
            loss_sb = small.tile([1, 1], F32, tag="losssb")
            nc.vector.tensor_copy(loss_sb[:], loss_ps[:])
            nc.sync.dma_start(out[:], loss_sb[:])

    nc.compile()
    return nc


def _get_nc():
    if "nc" not in _CACHE:
        _CACHE["nc"] = _build_nc()
    return _CACHE["nc"]


def _host_inputs(input, feature, sp):
    sp32 = np.asarray(sp).astype(np.int32).reshape(B, H, W)
    lo = (sp32 & 15).astype(ml_dtypes.bfloat16)
    ohhi = ((sp32 >> 4)[..., None] ==
            np.arange(16, dtype=np.int32)).astype(ml_dtypes.bfloat16)
    ohhi = np.ascontiguousarray(ohhi.reshape(B, H, W * 16))
    iota16 = np.broadcast_to(
        np.arange(16, dtype=np.float32)[None, :], (128, 16)
    ).astype(ml_dtypes.bfloat16)
    iota16 = np.ascontiguousarray(iota16)
    ryt = np.ascontiguousarray(_interp_matrix(H, FH).T)   # [64, 256]
    rxt = np.ascontiguousarray(_interp_matrix(W, FW).T)   # [64, 256]
    ones64 = np.full((C, 1), 1.0 / C, dtype=ml_dtypes.bfloat16)
    ones128 = np.ones((128, 1), dtype=np.float32)
    onesrow = np.ones((1, S), dtype=np.float32)
    xf = np.ascontiguousarray(np.asarray(input, dtype=np.float32))
    ff = np.ascontiguousarray(
        np.asarray(feature, dtype=np.float32)
        .reshape(B, C, FH * FW).astype(ml_dtypes.bfloat16))
    in_maps = []
    for b in range(B):
        in_maps.append({
            "x": xf[b],
            "f": ff[b],
            "lo": np.ascontiguousarray(lo[b]),
            "ohhi": ohhi[b],
            "iota16": iota16,
            "ryt": ryt,
            "rxt": rxt,
            "ones64": ones64,
            "ones128": ones128,
            "onesrow": onesrow,
        })
    return in_maps


def _run(inputs, trace=False, **kw):
    nc = _get_nc()
    in_maps = _host_inputs(inputs["input"], inputs["feature"], inputs["sp"])
    res = run_bass_kernel_spmd(nc, in_maps, core_ids=list(range(N_CORES)),
                               trace=trace, **kw)
    sums = np.array([res.results[i]["out"][0, 0] for i in range(N_CORES)],
                    dtype=np.float64)
    loss = (sums / float(S * S)).mean()
    return np.float32(loss), res


def kernel(**inputs) -> np.ndarray:
    loss, _ = _run(inputs, trace=False)
    return np.asarray(loss, dtype=np.float32)


# revision 19
# speedup vs baseline: 1.1583x; 1.0409x over previous
"""ConsistencyLoss kernel for 8 TRN2 NeuronCores (Bass/Tile).

loss = mean_b mean_{j,k} | |m1_j - m1_k| - |m2_j - m2_k| |
  m1 = per-segment means of channel-mean(input)
  m2 = per-segment means of channel-mean(bilinear_up(feature))

Sharding: data-parallel over batch B=8, one batch element per core.
Per-core pipeline (v2):
  - channel-mean of input on DVE via bf16 in-place add tree (h on partitions)
  - feature: channel-mean via ones-matmul, separable bilinear upsample via two
    small matmuls with host-built interpolation matrices
  - segment reduction, S=256=16*16 hi/lo one-hot decomposition, grouped
    cross-matrix matmuls: 8 pixel-columns share one [128,128] stationary
    (oh_hi), one N=384 moving block [A1|A2|oh_lo]x8 accumulates into a
    [128,384] PSUM tile; only the 8 diagonal [16,48] blocks are read out.
  - similarity-matrix L1 via K=2 outer-product matmuls + DVE abs/reduce
Host: casts sp to bf16 lo/hi planes, builds interp matrices, averages the
8 per-core sums.
"""

import sys

if "/opt/trn_rl_repo" not in sys.path:
    sys.path.insert(0, "/opt/trn_rl_repo")

import numpy as np
import ml_dtypes

import concourse.bacc as bacc
import concourse.mybir as mybir
import concourse.tile as tile
from concourse.bass_utils import run_bass_kernel_spmd

B, C, H, W = 8, 64, 256, 256
FH, FW = 64, 64
S = 256
N_CORES = 8

F32 = mybir.dt.float32
BF16 = mybir.dt.bfloat16

_CACHE = {}


def _interp_matrix(out_size: int, in_size: int) -> np.ndarray:
    """R [out,in]: bilinear align_corners row-interp matrix (float32)."""
    r = np.zeros((out_size, in_size), dtype=np.float64)
    ys = np.linspace(0.0, in_size - 1.0, out_size)
    y0 = np.floor(ys).astype(np.int64)
    y1 = np.minimum(y0 + 1, in_size - 1)
    wy = ys - y0
    for o in range(out_size):
        r[o, y0[o]] += 1.0 - wy[o]
        r[o, y1[o]] += wy[o]
    return r.astype(np.float32)


def _build_nc(dbg=False):
    nc = bacc.Bacc("TRN2", target_bir_lowering=False, debug=False,
                   num_devices=N_CORES)

    x = nc.dram_tensor("x", [C, H, W], F32, kind="ExternalInput").ap()
    f = nc.dram_tensor("f", [C, FH * FW], BF16, kind="ExternalInput").ap()
    lo = nc.dram_tensor("lo", [H, W], BF16, kind="ExternalInput").ap()
    ohhi_in = nc.dram_tensor("ohhi", [H, W * 16], BF16, kind="ExternalInput").ap()
    iota16 = nc.dram_tensor("iota16", [128, 16], BF16, kind="ExternalInput").ap()
    ryt = nc.dram_tensor("ryt", [FH, H], F32, kind="ExternalInput").ap()
    rxt = nc.dram_tensor("rxt", [FW, W], F32, kind="ExternalInput").ap()
    ones64 = nc.dram_tensor("ones64", [C, 1], BF16, kind="ExternalInput").ap()
    ones128 = nc.dram_tensor("ones128", [128, 1], F32, kind="ExternalInput").ap()
    onesrow = nc.dram_tensor("onesrow", [1, S], F32, kind="ExternalInput").ap()
    emat = nc.dram_tensor("emat", [128, 16], F32, kind="ExternalInput").ap()
    mask = nc.dram_tensor("mask", [128, 3 * 128], F32, kind="ExternalInput").ap()
    out = nc.dram_tensor("out", [1, 1], F32, kind="ExternalOutput").ap()
    if dbg:
        dbg_acc = nc.dram_tensor("dbg_acc", [16, 48], F32, kind="ExternalOutput").ap()
        dbg_px = nc.dram_tensor("dbg_px", [128, W], F32, kind="ExternalOutput").ap()
        dbg_m1 = nc.dram_tensor("dbg_m1", [16, 16], F32, kind="ExternalOutput").ap()
        dbg_m2 = nc.dram_tensor("dbg_m2", [16, 16], F32, kind="ExternalOutput").ap()

    CCH = 16          # channels per input DMA chunk
    NCC = C // CCH
    GW = 8            # pixel-columns per stationary group
    NG = W // GW      # groups per h-block

    with tile.TileContext(nc) as tc:
        with (
            tc.tile_pool(name="const", bufs=1) as const,
            tc.tile_pool(name="xin", bufs=4) as xin,
            tc.tile_pool(name="tree", bufs=1) as treep,
            tc.tile_pool(name="mov", bufs=2) as movp,
            tc.tile_pool(name="ohp", bufs=2) as ohp,
            tc.tile_pool(name="work", bufs=2) as work,
            tc.tile_pool(name="small", bufs=2) as small,
            tc.tile_pool(name="tail", bufs=1) as tailp,
            tc.tile_pool(name="ps1", bufs=1, space="PSUM") as ps1,
            tc.tile_pool(name="fmp", bufs=2, space="PSUM") as fmp,
            tc.tile_pool(name="psacc", bufs=1, space="PSUM") as psacc,
            tc.tile_pool(name="dps", bufs=1, space="PSUM") as dps,
        ):
            # ---- constants (SWDGE path; sync queue reserved for x) ----
            iota_sb = const.tile([128, 16], BF16, tag="iota")
            nc.gpsimd.dma_start(iota_sb[:], iota16[:])
            ryt_sb = const.tile([FH, H], F32, tag="ryt")
            nc.gpsimd.dma_start(ryt_sb[:], ryt[:])
            rxt_sb = const.tile([FW, W], F32, tag="rxt")
            nc.gpsimd.dma_start(rxt_sb[:], rxt[:])
            ones64_sb = const.tile([C, 1], BF16, tag="o64")
            nc.gpsimd.dma_start(ones64_sb[:], ones64[:])
            ones128_sb = const.tile([128, 1], F32, tag="o128")
            nc.gpsimd.dma_start(ones128_sb[:], ones128[:])
            emat_sb = const.tile([128, 16], F32, tag="emat")
            nc.gpsimd.dma_start(emat_sb[:], emat[:])
            mask_sb = const.tile([128, 3 * 128], F32, tag="mask")
            nc.gpsimd.dma_start(mask_sb[:], mask[:])

            # lh/rh ones-halves prefilled early
            lhrh = {}
            for nm in ("m1", "m2"):
                lh = const.tile([2, S], F32, tag=f"lh{nm}")
                nc.sync.dma_start(lh[1:2, :], onesrow[:])
                rh = const.tile([2, S], F32, tag=f"rh{nm}")
                nc.sync.dma_start(rh[0:1, :], onesrow[:])
                lhrh[nm] = (lh, rh)

            # ---- feature path: channel mean -> fm [64 h', 64 w'] ----
            fsb = const.tile([C, FH * FW], BF16, tag="fsb")
            nc.gpsimd.dma_start(fsb[:], f[:])
            fmsb = const.tile([FH, FW], F32, tag="fmsb")
            for i in range(8):
                fm_ps = fmp.tile([1, 512], F32, tag="fmps")
                nc.tensor.matmul(fm_ps[:], ones64_sb[:], fsb[:, i * 512:(i + 1) * 512])
                fmpart = small.tile([1, 512], F32, tag="fmpart")
                nc.scalar.copy(fmpart[:], fm_ps[:])
                nc.sync.dma_start(fmsb[i * 8:(i + 1) * 8, :], fmpart[:])

            # ---- bilinear upsample: fmup = Ry @ fm @ Rx^T ----
            t1_ps = ps1.tile([FW, H], F32, tag="t1ps")
            nc.tensor.matmul(t1_ps[:], fmsb[:], ryt_sb[:])
            t1_sb = const.tile([FW, H], F32, tag="t1sb")
            nc.scalar.copy(t1_sb[:], t1_ps[:])

            px2bf = []
            for hb in range(2):
                up_ps = ps1.tile([128, W], F32, tag="upps")
                nc.tensor.matmul(up_ps[:], t1_sb[:, hb * 128:(hb + 1) * 128],
                                 rxt_sb[:])
                p2 = work.tile([128, W], BF16, tag=f"px2bf{hb}")
                nc.scalar.copy(p2[:], up_ps[:])
                px2bf.append(p2)

            # ---- main loop over h-blocks ----
            acc_ps = psacc.tile([128, 3 * 128], F32, tag="acc")
            for hb in range(2):
                # input channel-sum via accumulate-DMA (CCE adds in DMA path):
                # 2 tiles x 4 overlaid 8-channel slabs -> [128, 8, 256] each,
                # then a small bf16 tree on DVE.
                parts = []
                for cc in range(NCC):
                    xt = xin.tile([128, CCH, W], F32, tag="xt")
                    nc.sync.dma_start(
                        xt[:],
                        x[cc * CCH:(cc + 1) * CCH,
                          hb * 128:(hb + 1) * 128, :].rearrange("c h w -> h c w"),
                    )
                    t8 = treep.tile([128, 8, W], BF16, tag=f"t8_{cc % 2}")
                    nc.vector.tensor_add(t8[:], xt[:, 0:8, :], xt[:, 8:16, :])
                    nc.vector.tensor_add(t8[:, 0:4, :], t8[:, 0:4, :],
                                         t8[:, 4:8, :])
                    nc.vector.tensor_add(t8[:, 0:2, :], t8[:, 0:2, :],
                                         t8[:, 2:4, :])
                    nc.vector.tensor_add(t8[:, 0:1, :], t8[:, 0:1, :],
                                         t8[:, 1:2, :])
                    if cc % 2 == 1:
                        psum_t = treep.tile([128, W], BF16, tag=f"pp{cc // 2}")
                        nc.vector.tensor_add(psum_t[:].unsqueeze(1),
                                             parts[-1][:, 0:1, :],
                                             t8[:, 0:1, :])
                        parts[-1] = psum_t
                    else:
                        parts.append(t8)
                px1 = work.tile([128, W], BF16, tag="px1bf")
                nc.vector.tensor_add(px1[:], parts[0][:], parts[1][:])
                if dbg and hb == 0:
                    pxf = tailp.tile([128, W], F32, tag="dbgpx")
                    nc.vector.tensor_copy(pxf[:], px1[:])
                    nc.sync.dma_start(dbg_px[:], pxf[:])

                # segment ids + prebuilt hi one-hot
                lo_sb = work.tile([128, W], BF16, tag="losb")
                nc.gpsimd.dma_start(lo_sb[:], lo[hb * 128:(hb + 1) * 128, :])
                ohhi = ohp.tile([128, W, 16], BF16, tag="ohhi")
                nc.gpsimd.dma_start(
                    ohhi[:],
                    ohhi_in[hb * 128:(hb + 1) * 128, :].rearrange(
                        "p (w j) -> p w j", j=16))

                # lo one-hot + A tiles in (w, j) layout, built per w-half so
                # the grouped matmuls overlap the builds
                mov = movp.tile([128, W, 48], BF16, tag="mov")
                ohlo = mov[:, :, 32:48]
                WH = W // 2
                for wh in range(2):
                    ws = slice(wh * WH, (wh + 1) * WH)
                    iota_b = iota_sb[:, :].unsqueeze(1).to_broadcast(
                        [128, WH, 16])
                    lo_b = lo_sb[:, ws].unsqueeze(2).to_broadcast(
                        [128, WH, 16])
                    px1_b = px1[:, ws].unsqueeze(2).to_broadcast(
                        [128, WH, 16])
                    px2_b = px2bf[hb][:, ws].unsqueeze(2).to_broadcast(
                        [128, WH, 16])
                    nc.vector.tensor_tensor(mov[:, ws, 32:48], iota_b, lo_b,
                                            op=mybir.AluOpType.is_equal)
                    nc.vector.tensor_tensor(mov[:, ws, 0:16],
                                            mov[:, ws, 32:48], px1_b,
                                            op=mybir.AluOpType.mult)
                    nc.vector.tensor_tensor(mov[:, ws, 16:32],
                                            mov[:, ws, 32:48], px2_b,
                                            op=mybir.AluOpType.mult)
                    ng2 = NG // 2
                    for g in range(wh * ng2, (wh + 1) * ng2):
                        lhs = ohhi[:, g * GW:(g + 1) * GW, :].rearrange(
                            "p w j -> p (w j)")
                        rhs = mov[:, g * GW:(g + 1) * GW, :].rearrange(
                            "p w j -> p (w j)")
                        nc.tensor.matmul(
                            acc_ps[:], lhs, rhs,
                            start=(hb == 0 and g == 0),
                            stop=(hb == 1 and g == NG - 1))

            # ---- diagonal extraction: mask off-diag blocks, sum row-blocks
            #      via E^T matmul, reduce slots on free axis ----
            acc_all = tailp.tile([128, 3 * 128], F32, tag="accall")
            nc.vector.tensor_tensor(acc_all[:], acc_ps[:], mask_sb[:],
                                    op=mybir.AluOpType.mult)
            ex_ps = ps1.tile([16, 3 * 128], F32, tag="expps")
            nc.tensor.matmul(ex_ps[:], emat_sb[:], acc_all[:])
            acc_sb = small.tile([16, 48], F32, tag="accsb")
            nc.vector.tensor_reduce(
                acc_sb[:], ex_ps.rearrange("p (s j) -> p j s", s=GW),
                axis=mybir.AxisListType.X, op=mybir.AluOpType.add)
            if dbg:
                nc.sync.dma_start(dbg_acc[:], acc_sb[:])

            # ---- m1/m2 [16,16] ----
            cntm = small.tile([16, 16], F32, tag="cntm")
            nc.vector.tensor_scalar_max(cntm[:], acc_sb[:, 32:48], 0.5)
            rc2 = small.tile([16, 16], F32, tag="rc2")
            nc.vector.reciprocal(rc2[:], cntm[:])
            rc1 = small.tile([16, 16], F32, tag="rc1")
            nc.vector.tensor_scalar_mul(rc1[:], rc2[:], 1.0 / C)
            m1 = small.tile([16, 16], F32, tag="m1")
            nc.vector.tensor_tensor(m1[:], acc_sb[:, 0:16], rc1[:],
                                    op=mybir.AluOpType.mult)
            m2 = small.tile([16, 16], F32, tag="m2")
            nc.vector.tensor_tensor(m2[:], acc_sb[:, 16:32], rc2[:],
                                    op=mybir.AluOpType.mult)
            if dbg:
                nc.sync.dma_start(dbg_m1[:], m1[:])
                nc.sync.dma_start(dbg_m2[:], m2[:])

            # rows+ones tiles for outer-product difference matmuls:
            # d[j,k] = m_j - m_k = [m_block; 1]^T @ [[1...]; [-m_row]]
            lhs_m, rhs_m = [], []
            for (msrc, nm) in ((m1, "m1"), (m2, "m2")):
                mneg = small.tile([16, 16], F32, tag=f"mneg{nm}")
                nc.vector.tensor_scalar_mul(mneg[:], msrc[:], -1.0)
                nc.sync.dma_start(lhrh[nm][0][0:1, :], msrc[:, :])
                nc.sync.dma_start(lhrh[nm][1][1:2, :], mneg[:, :])
                lhs_m.append(lhrh[nm][0])
                rhs_m.append(lhrh[nm][1])

            # ---- loss: sum_{j,k} ||m1_j-m1_k| - |m2_j-m2_k|| ----
            total = small.tile([128, 1], F32, tag="total")
            for jb in range(2):
                dps1 = dps.tile([128, S], F32, tag="dps1")
                nc.tensor.matmul(dps1[:], lhs_m[0][:, jb * 128:(jb + 1) * 128],
                                 rhs_m[0][:])
                dps2 = dps.tile([128, S], F32, tag="dps2")
                nc.tensor.matmul(dps2[:], lhs_m[1][:, jb * 128:(jb + 1) * 128],
                                 rhs_m[1][:])
                d1 = tailp.tile([128, S], F32, tag="d1")
                nc.vector.tensor_scalar(
                    d1[:].bitcast(mybir.dt.int32),
                    dps1[:].bitcast(mybir.dt.int32), 0x7FFFFFFF, None,
                    op0=mybir.AluOpType.bitwise_and)
                d2 = tailp.tile([128, S], F32, tag="d2")
                nc.vector.tensor_scalar(
                    d2[:].bitcast(mybir.dt.int32),
                    dps2[:].bitcast(mybir.dt.int32), 0x7FFFFFFF, None,
                    op0=mybir.AluOpType.bitwise_and)
                dd = tailp.tile([128, S], F32, tag="dd")
                nc.vector.tensor_tensor(dd[:], d1[:], d2[:],
                                        op=mybir.AluOpType.subtract)
                part = small.tile([128, 1], F32, tag=f"part{jb}")
                nc.vector.tensor_reduce(
                    part[:], dd[:], axis=mybir.AxisListType.X,
                    op=mybir.AluOpType.add, apply_absolute_value=True)
                if jb == 0:
                    tot0 = part
                else:
                    nc.vector.tensor_add(total[:], tot0[:], part[:])

            loss_ps = ps1.tile([1, 1], F32, tag="t1ps")
            nc.tensor.matmul(loss_ps[:], ones128_sb[:], total[:])
            loss_sb = small.tile([1, 1], F32, tag="losssb")
            nc.vector.tensor_copy(loss_sb[:], loss_ps[:])
            nc.sync.dma_start(out[:], loss_sb[:])

    nc.compile()
    return nc


def _get_nc():
    if "nc" not in _CACHE:
        _CACHE["nc"] = _build_nc()
    return _CACHE["nc"]


def _host_inputs(input, feature, sp):
    sp32 = np.asarray(sp).astype(np.int32).reshape(B, H, W)
    lo = (sp32 & 15).astype(ml_dtypes.bfloat16)
    ohhi = ((sp32 >> 4)[..., None] ==
            np.arange(16, dtype=np.int32)).astype(ml_dtypes.bfloat16)
    ohhi = np.ascontiguousarray(ohhi.reshape(B, H, W * 16))
    iota16 = np.broadcast_to(
        np.arange(16, dtype=np.float32)[None, :], (128, 16)
    ).astype(ml_dtypes.bfloat16)
    iota16 = np.ascontiguousarray(iota16)
    ryt = np.ascontiguousarray(_interp_matrix(H, FH).T)   # [64, 256]
    rxt = np.ascontiguousarray(_interp_matrix(W, FW).T)   # [64, 256]
    ones64 = np.full((C, 1), 1.0 / C, dtype=ml_dtypes.bfloat16)
    ones128 = np.ones((128, 1), dtype=np.float32)
    onesrow = np.ones((1, S), dtype=np.float32)
    emat = np.zeros((128, 16), dtype=np.float32)
    for i in range(8):
        for jh in range(16):
            emat[16 * i + jh, jh] = 1.0
    mask = np.zeros((128, 384), dtype=np.float32)
    for i in range(8):
        mask[16 * i:16 * (i + 1), 48 * i:48 * (i + 1)] = 1.0
    xf = np.ascontiguousarray(np.asarray(input, dtype=np.float32))
    ff = np.ascontiguousarray(
        np.asarray(feature, dtype=np.float32)
        .reshape(B, C, FH * FW).astype(ml_dtypes.bfloat16))
    in_maps = []
    for b in range(B):
        in_maps.append({
            "x": xf[b],
            "f": ff[b],
            "lo": np.ascontiguousarray(lo[b]),
            "ohhi": ohhi[b],
            "iota16": iota16,
            "ryt": ryt,
            "rxt": rxt,
            "ones64": ones64,
            "ones128": ones128,
            "onesrow": onesrow,
            "emat": emat,
            "mask": mask,
        })
    return in_maps


def _run(inputs, trace=False, **kw):
    nc = _get_nc()
    in_maps = _host_inputs(inputs["input"], inputs["feature"], inputs["sp"])
    res = run_bass_kernel_spmd(nc, in_maps, core_ids=list(range(N_CORES)),
                               trace=trace, **kw)
    sums = np.array([res.results[i]["out"][0, 0] for i in range(N_CORES)],
                    dtype=np.float64)
    loss = (sums / float(S * S)).mean()
    return np.float32(loss), res


def kernel(**inputs) -> np.ndarray:
    loss, _ = _run(inputs, trace=False)
    return np.asarray(loss, dtype=np.float32)


# revision 20
# speedup vs baseline: 1.2500x; 1.0792x over previous
"""ConsistencyLoss kernel for 8 TRN2 NeuronCores (Bass/Tile).

loss = mean_b mean_{j,k} | |m1_j - m1_k| - |m2_j - m2_k| |
  m1 = per-segment means of channel-mean(input)
  m2 = per-segment means of channel-mean(bilinear_up(feature))

Sharding: data-parallel over batch B=8, one batch element per core.
Per-core pipeline (v2):
  - channel-mean of input on DVE via bf16 in-place add tree (h on partitions)
  - feature: channel-mean via ones-matmul, separable bilinear upsample via two
    small matmuls with host-built interpolation matrices
  - segment reduction, S=256=16*16 hi/lo one-hot decomposition, grouped
    cross-matrix matmuls: 8 pixel-columns share one [128,128] stationary
    (oh_hi), one N=384 moving block [A1|A2|oh_lo]x8 accumulates into a
    [128,384] PSUM tile; only the 8 diagonal [16,48] blocks are read out.
  - similarity-matrix L1 via K=2 outer-product matmuls + DVE abs/reduce
Host: casts sp to bf16 lo/hi planes, builds interp matrices, averages the
8 per-core sums.
"""

import sys

if "/opt/trn_rl_repo" not in sys.path:
    sys.path.insert(0, "/opt/trn_rl_repo")

import numpy as np
import ml_dtypes

import concourse.bacc as bacc
import concourse.mybir as mybir
import concourse.tile as tile
from concourse.bass_utils import run_bass_kernel_spmd

B, C, H, W = 8, 64, 256, 256
FH, FW = 64, 64
S = 256
N_CORES = 8

F32 = mybir.dt.float32
BF16 = mybir.dt.bfloat16

_CACHE = {}


def _interp_matrix(out_size: int, in_size: int) -> np.ndarray:
    """R [out,in]: bilinear align_corners row-interp matrix (float32)."""
    r = np.zeros((out_size, in_size), dtype=np.float64)
    ys = np.linspace(0.0, in_size - 1.0, out_size)
    y0 = np.floor(ys).astype(np.int64)
    y1 = np.minimum(y0 + 1, in_size - 1)
    wy = ys - y0
    for o in range(out_size):
        r[o, y0[o]] += 1.0 - wy[o]
        r[o, y1[o]] += wy[o]
    return r.astype(np.float32)


def _build_nc(dbg=False):
    nc = bacc.Bacc("TRN2", target_bir_lowering=False, debug=False,
                   num_devices=N_CORES)

    x = nc.dram_tensor("x", [C, H, W], F32, kind="ExternalInput").ap()
    f = nc.dram_tensor("f", [C, FH * FW], BF16, kind="ExternalInput").ap()
    lo = nc.dram_tensor("lo", [H, W], BF16, kind="ExternalInput").ap()
    ohhi_in = nc.dram_tensor("ohhi", [H, W * 16], BF16, kind="ExternalInput").ap()
    iota16 = nc.dram_tensor("iota16", [128, 16], BF16, kind="ExternalInput").ap()
    ryt = nc.dram_tensor("ryt", [FH, H], F32, kind="ExternalInput").ap()
    rxt = nc.dram_tensor("rxt", [FW, W], F32, kind="ExternalInput").ap()
    ones64 = nc.dram_tensor("ones64", [C, 1], BF16, kind="ExternalInput").ap()
    ones128 = nc.dram_tensor("ones128", [128, 1], F32, kind="ExternalInput").ap()
    onesrow = nc.dram_tensor("onesrow", [1, S], F32, kind="ExternalInput").ap()
    emat = nc.dram_tensor("emat", [128, 16], F32, kind="ExternalInput").ap()
    mask = nc.dram_tensor("mask", [128, 3 * 128], F32, kind="ExternalInput").ap()
    out = nc.dram_tensor("out", [1, 1], F32, kind="ExternalOutput").ap()
    if dbg:
        dbg_acc = nc.dram_tensor("dbg_acc", [16, 48], F32, kind="ExternalOutput").ap()
        dbg_px = nc.dram_tensor("dbg_px", [128, W], F32, kind="ExternalOutput").ap()
        dbg_m1 = nc.dram_tensor("dbg_m1", [16, 16], F32, kind="ExternalOutput").ap()
        dbg_m2 = nc.dram_tensor("dbg_m2", [16, 16], F32, kind="ExternalOutput").ap()

    CCH = 16          # channels per input DMA chunk
    NCC = C // CCH
    GW = 8            # pixel-columns per stationary group
    NG = W // GW      # groups per h-block

    with tile.TileContext(nc) as tc:
        with (
            tc.tile_pool(name="const", bufs=1) as const,
            tc.tile_pool(name="xin", bufs=4) as xin,
            tc.tile_pool(name="tree", bufs=1) as treep,
            tc.tile_pool(name="mov", bufs=2) as movp,
            tc.tile_pool(name="ohp", bufs=2) as ohp,
            tc.tile_pool(name="work", bufs=2) as work,
            tc.tile_pool(name="small", bufs=2) as small,
            tc.tile_pool(name="tail", bufs=1) as tailp,
            tc.tile_pool(name="ps1", bufs=1, space="PSUM") as ps1,
            tc.tile_pool(name="fmp", bufs=2, space="PSUM") as fmp,
            tc.tile_pool(name="psacc", bufs=1, space="PSUM") as psacc,
            tc.tile_pool(name="dps", bufs=1, space="PSUM") as dps,
        ):
            # ---- constants (SWDGE path; sync queue reserved for x) ----
            iota_sb = const.tile([128, 16], BF16, tag="iota")
            nc.gpsimd.dma_start(iota_sb[:], iota16[:])
            ryt_sb = const.tile([FH, H], F32, tag="ryt")
            nc.gpsimd.dma_start(ryt_sb[:], ryt[:])
            rxt_sb = const.tile([FW, W], F32, tag="rxt")
            nc.gpsimd.dma_start(rxt_sb[:], rxt[:])
            ones64_sb = const.tile([C, 1], BF16, tag="o64")
            nc.gpsimd.dma_start(ones64_sb[:], ones64[:])
            ones128_sb = const.tile([128, 1], F32, tag="o128")
            nc.gpsimd.dma_start(ones128_sb[:], ones128[:])
            emat_sb = const.tile([128, 16], F32, tag="emat")
            nc.gpsimd.dma_start(emat_sb[:], emat[:])
            mask_sb = const.tile([128, 3 * 128], F32, tag="mask")
            nc.gpsimd.dma_start(mask_sb[:], mask[:])

            # lh/rh ones-halves prefilled early
            lhrh = {}
            for nm in ("m1", "m2"):
                lh = const.tile([2, S], F32, tag=f"lh{nm}")
                nc.sync.dma_start(lh[1:2, :], onesrow[:])
                rh = const.tile([2, S], F32, tag=f"rh{nm}")
                nc.sync.dma_start(rh[0:1, :], onesrow[:])
                lhrh[nm] = (lh, rh)

            # ---- feature path: channel mean -> fm [64 h', 64 w'] ----
            fsb = const.tile([C, FH * FW], BF16, tag="fsb")
            nc.gpsimd.dma_start(fsb[:], f[:])
            fmsb = const.tile([FH, FW], F32, tag="fmsb")
            for i in range(8):
                fm_ps = fmp.tile([1, 512], F32, tag="fmps")
                nc.tensor.matmul(fm_ps[:], ones64_sb[:], fsb[:, i * 512:(i + 1) * 512])
                fmpart = small.tile([1, 512], F32, tag="fmpart")
                nc.scalar.copy(fmpart[:], fm_ps[:])
                nc.sync.dma_start(fmsb[i * 8:(i + 1) * 8, :], fmpart[:])

            # ---- bilinear upsample: fmup = Ry @ fm @ Rx^T ----
            t1_ps = ps1.tile([FW, H], F32, tag="t1ps")
            nc.tensor.matmul(t1_ps[:], fmsb[:], ryt_sb[:])
            t1_sb = const.tile([FW, H], F32, tag="t1sb")
            nc.scalar.copy(t1_sb[:], t1_ps[:])

            px2bf = []
            for hb in range(2):
                up_ps = ps1.tile([128, W], F32, tag="upps")
                nc.tensor.matmul(up_ps[:], t1_sb[:, hb * 128:(hb + 1) * 128],
                                 rxt_sb[:])
                p2 = work.tile([128, W], BF16, tag=f"px2bf{hb}")
                nc.scalar.copy(p2[:], up_ps[:])
                px2bf.append(p2)

            # ---- main loop over h-blocks ----
            acc_ps = psacc.tile([128, 3 * 128], F32, tag="acc")
            for hb in range(2):
                # input channel-sum via accumulate-DMA (CCE adds in DMA path):
                # 2 tiles x 4 overlaid 8-channel slabs -> [128, 8, 256] each,
                # then a small bf16 tree on DVE.
                parts = []
                for cc in range(NCC):
                    xt = xin.tile([128, CCH, W], F32, tag="xt")
                    nc.sync.dma_start(
                        xt[:],
                        x[cc * CCH:(cc + 1) * CCH,
                          hb * 128:(hb + 1) * 128, :].rearrange("c h w -> h c w"),
                    )
                    t8 = treep.tile([128, 8, W], BF16, tag=f"t8_{cc % 2}")
                    nc.vector.tensor_add(t8[:], xt[:, 0:8, :], xt[:, 8:16, :])
                    nc.vector.tensor_add(t8[:, 0:4, :], t8[:, 0:4, :],
                                         t8[:, 4:8, :])
                    nc.vector.tensor_add(t8[:, 0:2, :], t8[:, 0:2, :],
                                         t8[:, 2:4, :])
                    nc.vector.tensor_add(t8[:, 0:1, :], t8[:, 0:1, :],
                                         t8[:, 1:2, :])
                    if cc % 2 == 1:
                        psum_t = treep.tile([128, W], BF16, tag=f"pp{cc // 2}")
                        nc.vector.tensor_add(psum_t[:].unsqueeze(1),
                                             parts[-1][:, 0:1, :],
                                             t8[:, 0:1, :])
                        parts[-1] = psum_t
                    else:
                        parts.append(t8)
                px1 = work.tile([128, W], BF16, tag="px1bf")
                nc.vector.tensor_add(px1[:], parts[0][:], parts[1][:])
                if dbg and hb == 0:
                    pxf = tailp.tile([128, W], F32, tag="dbgpx")
                    nc.vector.tensor_copy(pxf[:], px1[:])
                    nc.sync.dma_start(dbg_px[:], pxf[:])

                # segment ids + prebuilt hi one-hot
                lo_sb = work.tile([128, W], BF16, tag="losb")
                nc.gpsimd.dma_start(lo_sb[:], lo[hb * 128:(hb + 1) * 128, :])
                ohhi = ohp.tile([128, W, 16], BF16, tag="ohhi")
                nc.gpsimd.dma_start(
                    ohhi.rearrange("p w j -> p (w j)"),
                    ohhi_in[hb * 128:(hb + 1) * 128, :])

                # lo one-hot + A tiles in (w, j) layout, built per w-half so
                # the grouped matmuls overlap the builds
                mov = movp.tile([128, W, 48], BF16, tag="mov")
                ohlo = mov[:, :, 32:48]
                WH = W // 2
                for wh in range(2):
                    ws = slice(wh * WH, (wh + 1) * WH)
                    iota_b = iota_sb[:, :].unsqueeze(1).to_broadcast(
                        [128, WH, 16])
                    lo_b = lo_sb[:, ws].unsqueeze(2).to_broadcast(
                        [128, WH, 16])
                    px1_b = px1[:, ws].unsqueeze(2).to_broadcast(
                        [128, WH, 16])
                    px2_b = px2bf[hb][:, ws].unsqueeze(2).to_broadcast(
                        [128, WH, 16])
                    nc.vector.tensor_tensor(mov[:, ws, 32:48], iota_b, lo_b,
                                            op=mybir.AluOpType.is_equal)
                    nc.vector.tensor_tensor(mov[:, ws, 0:16],
                                            mov[:, ws, 32:48], px1_b,
                                            op=mybir.AluOpType.mult)
                    nc.vector.tensor_tensor(mov[:, ws, 16:32],
                                            mov[:, ws, 32:48], px2_b,
                                            op=mybir.AluOpType.mult)
                    ng2 = NG // 2
                    for g in range(wh * ng2, (wh + 1) * ng2):
                        lhs = ohhi[:, g * GW:(g + 1) * GW, :].rearrange(
                            "p w j -> p (w j)")
                        rhs = mov[:, g * GW:(g + 1) * GW, :].rearrange(
                            "p w j -> p (w j)")
                        nc.tensor.matmul(
                            acc_ps[:], lhs, rhs,
                            start=(hb == 0 and g == 0),
                            stop=(hb == 1 and g == NG - 1))

            # ---- diagonal extraction: mask off-diag blocks, sum row-blocks
            #      via E^T matmul, reduce slots on free axis ----
            acc_all = tailp.tile([128, 3 * 128], F32, tag="accall")
            nc.vector.tensor_tensor(acc_all[:], acc_ps[:], mask_sb[:],
                                    op=mybir.AluOpType.mult)
            ex_ps = ps1.tile([16, 3 * 128], F32, tag="expps")
            nc.tensor.matmul(ex_ps[:], emat_sb[:], acc_all[:])
            acc_sb = small.tile([16, 48], F32, tag="accsb")
            nc.vector.tensor_reduce(
                acc_sb[:], ex_ps.rearrange("p (s j) -> p j s", s=GW),
                axis=mybir.AxisListType.X, op=mybir.AluOpType.add)
            if dbg:
                nc.sync.dma_start(dbg_acc[:], acc_sb[:])

            # ---- m1/m2 [16,16] ----
            cntm = small.tile([16, 16], F32, tag="cntm")
            nc.vector.tensor_scalar_max(cntm[:], acc_sb[:, 32:48], 0.5)
            rc2 = small.tile([16, 16], F32, tag="rc2")
            nc.vector.reciprocal(rc2[:], cntm[:])
            rc1 = small.tile([16, 16], F32, tag="rc1")
            nc.vector.tensor_scalar_mul(rc1[:], rc2[:], 1.0 / C)
            m1 = small.tile([16, 16], F32, tag="m1")
            nc.vector.tensor_tensor(m1[:], acc_sb[:, 0:16], rc1[:],
                                    op=mybir.AluOpType.mult)
            m2 = small.tile([16, 16], F32, tag="m2")
            nc.vector.tensor_tensor(m2[:], acc_sb[:, 16:32], rc2[:],
                                    op=mybir.AluOpType.mult)
            if dbg:
                nc.sync.dma_start(dbg_m1[:], m1[:])
                nc.sync.dma_start(dbg_m2[:], m2[:])

            # rows+ones tiles for outer-product difference matmuls:
            # d[j,k] = m_j - m_k = [m_block; 1]^T @ [[1...]; [-m_row]]
            lhs_m, rhs_m = [], []
            for (msrc, nm) in ((m1, "m1"), (m2, "m2")):
                mneg = small.tile([16, 16], F32, tag=f"mneg{nm}")
                nc.vector.tensor_scalar_mul(mneg[:], msrc[:], -1.0)
                nc.sync.dma_start(lhrh[nm][0][0:1, :], msrc[:, :])
                nc.sync.dma_start(lhrh[nm][1][1:2, :], mneg[:, :])
                lhs_m.append(lhrh[nm][0])
                rhs_m.append(lhrh[nm][1])

            # ---- loss: sum_{j,k} ||m1_j-m1_k| - |m2_j-m2_k|| ----
            total = small.tile([128, 1], F32, tag="total")
            for jb in range(2):
                dps1 = dps.tile([128, S], F32, tag="dps1")
                nc.tensor.matmul(dps1[:], lhs_m[0][:, jb * 128:(jb + 1) * 128],
                                 rhs_m[0][:])
                dps2 = dps.tile([128, S], F32, tag="dps2")
                nc.tensor.matmul(dps2[:], lhs_m[1][:, jb * 128:(jb + 1) * 128],
                                 rhs_m[1][:])
                d1 = tailp.tile([128, S], F32, tag="d1")
                nc.vector.tensor_scalar(
                    d1[:].bitcast(mybir.dt.int32),
                    dps1[:].bitcast(mybir.dt.int32), 0x7FFFFFFF, None,
                    op0=mybir.AluOpType.bitwise_and)
                d2 = tailp.tile([128, S], F32, tag="d2")
                nc.vector.tensor_scalar(
                    d2[:].bitcast(mybir.dt.int32),
                    dps2[:].bitcast(mybir.dt.int32), 0x7FFFFFFF, None,
                    op0=mybir.AluOpType.bitwise_and)
                dd = tailp.tile([128, S], F32, tag="dd")
                nc.vector.tensor_tensor(dd[:], d1[:], d2[:],
                                        op=mybir.AluOpType.subtract)
                part = small.tile([128, 1], F32, tag=f"part{jb}")
                nc.vector.tensor_reduce(
                    part[:], dd[:], axis=mybir.AxisListType.X,
                    op=mybir.AluOpType.add, apply_absolute_value=True)
                if jb == 0:
                    tot0 = part
                else:
                    nc.vector.tensor_add(total[:], tot0[:], part[:])

            loss_ps = ps1.tile([1, 1], F32, tag="t1ps")
            nc.tensor.matmul(loss_ps[:], ones128_sb[:], total[:])
            loss_sb = small.tile([1, 1], F32, tag="losssb")
            nc.vector.tensor_copy(loss_sb[:], loss_ps[:])
            nc.sync.dma_start(out[:], loss_sb[:])

    nc.compile()
    return nc


def _get_nc():
    if "nc" not in _CACHE:
        _CACHE["nc"] = _build_nc()
    return _CACHE["nc"]


def _host_inputs(input, feature, sp):
    sp32 = np.asarray(sp).astype(np.int32).reshape(B, H, W)
    lo = (sp32 & 15).astype(ml_dtypes.bfloat16)
    ohhi = ((sp32 >> 4)[..., None] ==
            np.arange(16, dtype=np.int32)).astype(ml_dtypes.bfloat16)
    ohhi = np.ascontiguousarray(ohhi.reshape(B, H, W * 16))
    iota16 = np.broadcast_to(
        np.arange(16, dtype=np.float32)[None, :], (128, 16)
    ).astype(ml_dtypes.bfloat16)
    iota16 = np.ascontiguousarray(iota16)
    ryt = np.ascontiguousarray(_interp_matrix(H, FH).T)   # [64, 256]
    rxt = np.ascontiguousarray(_interp_matrix(W, FW).T)   # [64, 256]
    ones64 = np.full((C, 1), 1.0 / C, dtype=ml_dtypes.bfloat16)
    ones128 = np.ones((128, 1), dtype=np.float32)
    onesrow = np.ones((1, S), dtype=np.float32)
    emat = np.zeros((128, 16), dtype=np.float32)
    for i in range(8):
        for jh in range(16):
            emat[16 * i + jh, jh] = 1.0
    mask = np.zeros((128, 384), dtype=np.float32)
    for i in range(8):
        mask[16 * i:16 * (i + 1), 48 * i:48 * (i + 1)] = 1.0
    xf = np.ascontiguousarray(np.asarray(input, dtype=np.float32))
    ff = np.ascontiguousarray(
        np.asarray(feature, dtype=np.float32)
        .reshape(B, C, FH * FW).astype(ml_dtypes.bfloat16))
    in_maps = []
    for b in range(B):
        in_maps.append({
            "x": xf[b],
            "f": ff[b],
            "lo": np.ascontiguousarray(lo[b]),
            "ohhi": ohhi[b],
            "iota16": iota16,
            "ryt": ryt,
            "rxt": rxt,
            "ones64": ones64,
            "ones128": ones128,
            "onesrow": onesrow,
            "emat": emat,
            "mask": mask,
        })
    return in_maps


def _run(inputs, trace=False, **kw):
    nc = _get_nc()
    in_maps = _host_inputs(inputs["input"], inputs["feature"], inputs["sp"])
    res = run_bass_kernel_spmd(nc, in_maps, core_ids=list(range(N_CORES)),
                               trace=trace, **kw)
    sums = np.array([res.results[i]["out"][0, 0] for i in range(N_CORES)],
                    dtype=np.float64)
    loss = (sums / float(S * S)).mean()
    return np.float32(loss), res


def kernel(**inputs) -> np.ndarray:
    loss, _ = _run(inputs, trace=False)
    return np.asarray(loss, dtype=np.float32)
